# revision 1
# baseline (speedup 1.0000x reference)
"""Trainium2 Bass kernel for nn_Attention_38233798869191.

Full multi-head attention layer (B=2, S=2048, D=1024, H=16, dh=64) with the
reference's "faithful to original" reshape quirk, sharded over 8 NeuronCores
by splitting heads (tensor parallel): core c owns heads {2c, 2c+1}.

Per-core dataflow (everything transposed: feature dim on SBUF partitions):
  xT [1024, 4096]     (host-pretransposed x, shared by all cores)
  qT = (WqT_c.T @ xT) * SCALE   [128, 4096]   (2 heads x 64 dims)
  kT, vT likewise.
  v natural [sk, dh] built from vT via TensorE transposes, with an extra
  ones-column so the p@v matmul also produces the softmax denominators.
  Per (b, head): scoresT[sk, sq] = kT.T @ qT; p = exp(scoresT) (no max
  subtraction -- scores are O(1) by construction); oT' = [v | 1].T @ p
  accumulated over sk chunks in PSUM; transpose back, normalize by the
  denominator row, write o natural [2048, 64] to a DRAM scratch.
  The reference's o.reshape(B, D, S).swapaxes trick means the output
  projection is y[b].T = Wo @ M[b] where M[b][h*64:(h+1)*64] is simply
  o_natural[b,h] reinterpreted as [64, 2048] (contiguous reshape), so the
  scratch is DMA'd back as [64, 2048] rows of M. Each core computes the
  partial y[b].T = Wo[:, c*128:(c+1)*128] @ M_c[b]; partials are summed on
  the host (the tensor-parallel all-reduce) and transposed back.
"""

import os
import sys

import numpy as np

for _p in ("/opt/trn_rl_repo", "/root/.axon_site/_ro/trn_rl_repo"):
    if os.path.isdir(_p) and _p not in sys.path:
        sys.path.insert(0, _p)

B, S, D, H, DH = 2, 2048, 1024, 16, 64
NSEQ = B * S  # 4096
SCALE = 1.0 / float(np.sqrt(DH))
N_CORES = 8
P = 128

# "f32r": fp32 data, matmuls in float32r (full-rate fp32 mode)
# "bf16": bf16 data + matmuls
# "f32": fp32 data, plain fp32 matmuls (4x slower, reference mode)
DTYPE_MODE = os.environ.get("KERNEL_DTYPE_MODE", "f32r")


def _build_nc(mode, reps=1):
    import concourse.bass as bass  # noqa: F401
    import concourse.mybir as mybir
    import concourse.tile as tile
    from concourse import bacc
    from concourse.masks import make_identity

    f32 = mybir.dt.float32
    if mode == "bf16":
        elt = mybir.dt.bfloat16
        mmdt = mybir.dt.bfloat16
    elif mode == "f32r":
        # float32r tiles everywhere: the BIR verifier requires every producer
        # feeding an fp32r matmul to emit fp32r-rounded values.
        elt = mybir.dt.float32r
        mmdt = mybir.dt.float32r
    else:
        elt = f32
        mmdt = f32
    AF = mybir.ActivationFunctionType

    nc = bacc.Bacc(
        "TRN2",
        target_bir_lowering=False,
        debug=False,
        num_devices=N_CORES,
    )

    xT = nc.dram_tensor("xT", [D, NSEQ], elt, kind="ExternalInput")
    wqT = nc.dram_tensor("wqT", [D, P], elt, kind="ExternalInput")
    wkT = nc.dram_tensor("wkT", [D, P], elt, kind="ExternalInput")
    wvT = nc.dram_tensor("wvT", [D, P], elt, kind="ExternalInput")
    woT = nc.dram_tensor("woT", [P, D], elt, kind="ExternalInput")
    bqs = nc.dram_tensor("bqs", [P, 1], f32, kind="ExternalInput")  # bq*SCALE
    bk = nc.dram_tensor("bk", [P, 1], f32, kind="ExternalInput")
    bv = nc.dram_tensor("bv", [P, 1], f32, kind="ExternalInput")
    ypT = nc.dram_tensor("ypT", [B, D, S], f32, kind="ExternalOutput")
    osc = nc.dram_tensor("osc", [2 * 2, S, DH], elt)  # o natural per (b, hl)

    # DRAM views
    # k global = ko*512 + ks*128 + p
    xTv = xT.ap().rearrange("(ko ks p) s -> ko p ks s", ks=4, p=P)
    oscM = osc.ap().rearrange("h (r k) d -> h r (k d)", r=64)  # [4, 64, 2048]

    def wview(w):
        return w.ap().rearrange("(kc p) m -> p kc m", p=P)  # [128, 8, 128]

    with tile.TileContext(nc) as tc:
        with tc.tile_pool(name="persist", bufs=1) as pp:
            # persistent SBUF tensors
            w_sb = {}
            for name, w in (("q", wqT), ("k", wkT), ("v", wvT)):
                w_sb[name] = pp.tile([P, 8, P], elt, tag=f"w{name}", name=f"w{name}")
                nc.sync.dma_start(w_sb[name][:], wview(w))
            woT_sb = pp.tile([P, D], elt, tag="wo", name="wo")
            nc.sync.dma_start(woT_sb[:], woT.ap())
            bias_sb = {}
            for name, bt in (("q", bqs), ("k", bk), ("v", bv)):
                bias_sb[name] = pp.tile([P, 1], f32, tag=f"b{name}", name=f"b{name}")
                nc.sync.dma_start(bias_sb[name][:], bt.ap())
            # vT only feeds TensorE transposes (not fp32r matmuls), so in
            # f32r mode it stays plain f32 (memset/make_identity/transpose
            # all dislike f32r); rounding to f32r happens at the v_nat copy.
            vt_dt = f32 if mode == "f32r" else elt
            qT_sb = pp.tile([P, NSEQ], elt, tag="qT", name="qT")
            kT_sb = pp.tile([P, NSEQ], elt, tag="kT", name="kT")
            vT_sb = pp.tile([P, NSEQ], vt_dt, tag="vT", name="vT")
            proj_sb = {"q": qT_sb, "k": kT_sb, "v": vT_sb}
            M_sb = [pp.tile([P, S], elt, tag=f"M{b}", name=f"M{b}") for b in range(B)]
            v_nat = [pp.tile([P, 16, 72], elt, tag=f"vn{i}", name=f"vn{i}") for i in range(4)]
            # stacked identity: I64 in both partition halves, so transposes of
            # operands based at partition 0 or 64 both have a matching rhs
            id2 = pp.tile([P, 64], vt_dt, tag="id2", name="id2")
            make_identity(nc, id2[0:64, :])
            make_identity(nc, id2[64:128, :])
            id_f32 = pp.tile([P, P], f32, tag="id_f32", name="id_f32")
            make_identity(nc, id_f32[:])
            ones_sb = pp.tile([P, 16], f32, tag="ones", name="ones")
            nc.vector.memset(ones_sb[:], 1.0)

            for _rep in range(reps):
                with (
                    tc.tile_pool(name="xin", bufs=6) as xpool,
                    # one shared PSUM budget (8 banks):
                    #   sps: 2 x [128,1024] f32   = 4 banks (scores, proj, yproj)
                    #   ops: 1 x [65,1024] f32    = 2 banks (o accumulator)
                    #   tps: 2 x [128,72]         = 2 banks (all transposes)
                    tc.tile_pool(name="sps", bufs=2, space="PSUM") as sps,
                    tc.tile_pool(name="ops", bufs=1, space="PSUM") as ops,
                    tc.tile_pool(name="tps", bufs=2, space="PSUM") as tps,
                    tc.tile_pool(name="ptp", bufs=4) as ptp,
                    tc.tile_pool(name="otp", bufs=2) as otp,
                    tc.tile_pool(name="obp", bufs=2) as obp,
                    tc.tile_pool(name="rcp", bufs=3) as rcp,
                    tc.tile_pool(name="ysb", bufs=4) as ysbp,
                ):

                    def proj_batch(bi):
                        """q/k/v projections for batch bi's sequence columns,
                        then v_nat build for its two heads."""
                        for sq in range(bi * 4, bi * 4 + 4):
                            s12 = sps.tile([P, 1024], f32, tag="s", name="s")
                            s3 = sps.tile([P, 1024], f32, tag="s", name="s")
                            acc = {
                                "q": s12[:, 0:512],
                                "k": s12[:, 512:1024],
                                "v": s3[:, 0:512],
                            }
                            for ko in range(2):
                                x_sb = xpool.tile(
                                    [P, 4, 512], elt, tag="x", name="x"
                                )
                                for xh in range(2):
                                    nc.sync.dma_start(
                                        x_sb[:, xh * 2 : (xh + 1) * 2, :],
                                        xTv[
                                            ko,
                                            :,
                                            xh * 2 : (xh + 1) * 2,
                                            sq * 512 : (sq + 1) * 512,
                                        ],
                                    )
                                for n in "qkv":
                                    for ks in range(4):
                                        nc.tensor.matmul(
                                            acc[n],
                                            w_sb[n][
                                                :, ko * 4 + ks, :
                                            ].bitcast(mmdt),
                                            x_sb[:, ks, :].bitcast(mmdt),
                                            start=(ko == 0 and ks == 0),
                                            stop=(ko == 1 and ks == 3),
                                        )
                            sl = slice(sq * 512, (sq + 1) * 512)
                            nc.scalar.activation(
                                qT_sb[:, sl], acc["q"], AF.Identity,
                                bias=bias_sb["q"][:], scale=SCALE,
                            )
                            nc.scalar.activation(
                                kT_sb[:, sl], acc["k"], AF.Identity,
                                bias=bias_sb["k"][:],
                            )
                            nc.scalar.activation(
                                vT_sb[:, sl], acc["v"], AF.Identity,
                                bias=bias_sb["v"][:],
                            )
                        # v natural (+ ones column) for this batch's heads
                        for hl in range(2):
                            bh = bi * 2 + hl
                            hsl = slice(hl * 64, (hl + 1) * 64)
                            nc.vector.tensor_copy(
                                v_nat[bh][:, :, 64:65], ones_sb[:, :, None]
                            )
                            for t in range(16):
                                c0 = bi * S + t * P
                                pt = tps.tile([P, 72], vt_dt, tag="t2", name="t2")
                                nc.tensor.transpose(
                                    pt[:, :64],
                                    vT_sb[hsl, c0 : c0 + P],
                                    id2[hsl, :],
                                )
                                nc.vector.tensor_copy(
                                    v_nat[bh][:, t, 0:64], pt[:, :64]
                                )

                    def normalize_half(bh, sqh, ot):
                        """transpose back + divide by denominators + store o."""
                        ob = obp.tile([P, 8, DH], elt, tag="ob", name="ob")
                        for tb in range(8):
                            pt2 = tps.tile([P, 72], f32, tag="t2", name="t2")
                            nc.tensor.transpose(
                                pt2[:, :65],
                                ot[:, tb * P : (tb + 1) * P],
                                id_f32[:65, :65],
                            )
                            rc = rcp.tile([P, 1], f32, tag="rc", name="rc")
                            nc.vector.reciprocal(rc[:], pt2[:, 64:65])
                            nc.vector.tensor_scalar_mul(
                                ob[:, tb, :], pt2[:, 0:64], rc[:]
                            )
                        s0 = sqh * 1024
                        nc.sync.dma_start(
                            osc.ap()[bh, s0 : s0 + 1024, :].rearrange(
                                "(t p) d -> p t d", p=P
                            ),
                            ob[:],
                        )

                    def attention_head(b, hl, interleave_norm=False, extra_work=None):
                        bh = b * 2 + hl
                        hsl = slice(hl * 64, (hl + 1) * 64)
                        pending = []
                        for sqh in range(2):  # halves of 1024 queries
                            sq0 = b * S + sqh * 1024
                            po = ops.tile(
                                [65, 1024], f32, tag="oacc", name="oacc"
                            )

                            def emit_pv(kc, ptile):
                                for half in range(2):
                                    nc.tensor.matmul(
                                        po[:, half * 512 : (half + 1) * 512],
                                        v_nat[bh][:, kc, 0:65].bitcast(mmdt),
                                        ptile[
                                            :, half * 512 : (half + 1) * 512
                                        ].bitcast(mmdt),
                                        start=(kc == 0),
                                        stop=(kc == 15),
                                    )

                            # software-pipelined: pv lags one kc so the next
                            # qk runs on PE while ACT does exp
                            pending_pv = []
                            for kc in range(16):
                                k0 = b * S + kc * P
                                ps2 = sps.tile(
                                    [P, 1024], f32, tag="s", name="s"
                                )
                                for half in range(2):
                                    nc.tensor.matmul(
                                        ps2[:, half * 512 : (half + 1) * 512],
                                        kT_sb[hsl, k0 : k0 + P].bitcast(mmdt),
                                        qT_sb[
                                            hsl,
                                            sq0 + half * 512 : sq0
                                            + (half + 1) * 512,
                                        ].bitcast(mmdt),
                                        start=True,
                                        stop=True,
                                    )
                                ptile = ptp.tile(
                                    [P, 1024], elt, tag="pt", name="pt"
                                )
                                nc.scalar.activation(ptile[:], ps2[:], AF.Exp)
                                pending_pv.append((kc, ptile))
                                if len(pending_pv) > 2:
                                    emit_pv(*pending_pv.pop(0))
                                if extra_work:
                                    extra_work.pop(0)()
                            for args in pending_pv:
                                emit_pv(*args)

                            ot = otp.tile([65, 1024], f32, tag="ot", name="ot")
                            nc.vector.tensor_copy(ot[:], po[:])
                            if interleave_norm:
                                normalize_half(bh, sqh, ot)
                            else:
                                pending.append((bh, sqh, ot))

                        for args in pending:
                            normalize_half(*args)
                        # M rows for this head
                        nc.sync.dma_start(
                            M_sb[b][hsl.start + 0 : hsl.start + 64, :],
                            oscM[bh],
                        )

                    _ysb_live = {}

                    def outproj_unit(b, mo, nh, evict):
                        if nh == 0:
                            _ysb_live[(b, mo)] = ysbp.tile(
                                [P, 2 * 1024], f32, tag="y", name="y"
                            )
                        ysb = _ysb_live[(b, mo)]
                        py = sps.tile([P, 1024], f32, tag="s", name="s")
                        for half in range(2):
                            n0 = nh * 1024 + half * 512
                            nc.tensor.matmul(
                                py[:, half * 512 : (half + 1) * 512],
                                woT_sb[:, mo * P : (mo + 1) * P].bitcast(mmdt),
                                M_sb[b][:, n0 : n0 + 512].bitcast(mmdt),
                                start=True,
                                stop=True,
                            )
                        dst = ysb[:, nh * 1024 : (nh + 1) * 1024]
                        if evict == "dve" or (evict == "alt" and nh == 0):
                            nc.vector.tensor_copy(dst, py[:])
                        else:
                            nc.scalar.copy(dst, py[:])
                        if nh == 1:
                            nc.sync.dma_start(
                                ypT.ap()[b, mo * P : (mo + 1) * P, :], ysb[:]
                            )

                    def outproj_units(b, evict):
                        return [
                            (lambda mo=mo, nh=nh: outproj_unit(b, mo, nh, evict))
                            for mo in range(8)
                            for nh in range(2)
                        ]

                    def outproj(b, evict):
                        for u in outproj_units(b, evict):
                            u()

                    proj_batch(0)
                    attention_head(0, 0)
                    proj_batch(1)
                    attention_head(0, 1)
                    attention_head(1, 0)
                    attention_head(
                        1, 1, interleave_norm=True,
                        extra_work=outproj_units(0, "dve"),
                    )
                    outproj(1, evict="alt")

    nc.compile()
    return nc


_CACHE = {}


def _np_elt(mode):
    if mode == "bf16":
        import ml_dtypes

        return ml_dtypes.bfloat16
    return np.float32


def _get_runner(mode, reps=1):
    """Build (once) the compiled kernel + a persistent jitted executor."""
    key = (mode, reps)
    if key in _CACHE:
        return _CACHE[key]

    import jax
    import jax.numpy as jnp  # noqa: F401
    from jax.sharding import Mesh, PartitionSpec
    from jax.experimental.shard_map import shard_map
    import concourse.mybir as mybir
    from concourse import bass2jax

    nc = _build_nc(mode, reps)
    bass2jax.install_neuronx_cc_hook()

    partition_name = (
        nc.partition_id_tensor.name if nc.partition_id_tensor else None
    )
    in_names = []
    out_names = []
    out_avals = []
    for alloc in nc.m.functions[0].allocations:
        if not isinstance(alloc, mybir.MemoryLocationSet):
            continue
        name = alloc.memorylocations[0].name
        if alloc.kind == "ExternalInput":
            if name != partition_name:
                in_names.append(name)
        elif alloc.kind == "ExternalOutput":
            out_names.append(name)
            shape = tuple(alloc.tensor_shape)
            dtype = mybir.dt.np(alloc.dtype)
            out_avals.append(jax.core.ShapedArray(shape, dtype))
    n_params = len(in_names)
    n_outs = len(out_avals)
    all_in_names = list(in_names) + list(out_names)
    if partition_name is not None:
        all_in_names.append(partition_name)
    all_in_names = tuple(all_in_names)

    def _body(*args):
        operands = list(args)
        if partition_name is not None:
            operands.append(bass2jax.partition_id_tensor())
        outs = bass2jax._bass_exec_p.bind(
            *operands,
            out_avals=tuple(out_avals),
            in_names=all_in_names,
            out_names=tuple(out_names),
            lowering_input_output_aliases=(),
            sim_require_finite=True,
            sim_require_nnan=True,
            nc=nc,
        )
        return tuple(outs)

    devices = jax.devices()[:N_CORES]
    mesh = Mesh(np.asarray(devices), ("core",))
    in_specs = (PartitionSpec("core"),) * (n_params + n_outs)
    out_specs = (PartitionSpec("core"),) * n_outs
    donate = tuple(range(n_params, n_params + n_outs))
    sharded = jax.jit(
        shard_map(
            _body, mesh=mesh, in_specs=in_specs, out_specs=out_specs,
            check_rep=False,
        ),
        donate_argnums=donate,
        keep_unused=True,
    )

    zero_out_shapes = [
        ((N_CORES * a.shape[0],) + tuple(a.shape[1:]), a.dtype)
        for a in out_avals
    ]

    def execute(in_maps):
        concat_in = [
            np.concatenate([np.asarray(m[name]) for m in in_maps], axis=0)
            for name in in_names
        ]
        concat_zeros = [np.zeros(s, d) for s, d in zero_out_shapes]
        out_arrs = sharded(*concat_in, *concat_zeros)
        out_arrs = [np.asarray(o) for o in out_arrs]
        return [
            {
                name: out_arrs[i].reshape(
                    N_CORES, *out_avals[i].shape
                )[c]
                for i, name in enumerate(out_names)
            }
            for c in range(N_CORES)
        ]

    execute.in_names = in_names
    execute.out_names = out_names
    execute.out_avals = out_avals
    execute.n_params = n_params
    execute.body = _body
    execute.mesh = mesh
    execute.zero_out_shapes = zero_out_shapes
    _CACHE[key] = execute
    return execute


def make_in_maps(x, Wq, bq, Wk, bk, Wv, bv, Wo, bo, mode=None):
    mode = mode or DTYPE_MODE
    ne = _np_elt(mode)
    x = np.asarray(x, np.float32)
    xT = np.ascontiguousarray(x.reshape(NSEQ, D).T).astype(ne)
    in_maps = []
    for c in range(N_CORES):
        sl = slice(c * P, (c + 1) * P)
        in_maps.append(
            {
                "xT": xT,
                "wqT": np.ascontiguousarray(np.asarray(Wq)[sl, :].T).astype(ne),
                "wkT": np.ascontiguousarray(np.asarray(Wk)[sl, :].T).astype(ne),
                "wvT": np.ascontiguousarray(np.asarray(Wv)[sl, :].T).astype(ne),
                "woT": np.ascontiguousarray(np.asarray(Wo)[:, sl].T).astype(ne),
                "bqs": (np.asarray(bq, np.float32)[sl] * SCALE).reshape(P, 1),
                "bk": np.asarray(bk, np.float32)[sl].reshape(P, 1).copy(),
                "bv": np.asarray(bv, np.float32)[sl].reshape(P, 1).copy(),
            }
        )
    return in_maps


def kernel(x, Wq, bq, Wk, bk, Wv, bv, Wo, bo):
    mode = DTYPE_MODE
    execute = _get_runner(mode)
    in_maps = make_in_maps(x, Wq, bq, Wk, bk, Wv, bv, Wo, bo, mode)
    results = execute(in_maps)
    ysum = np.zeros((B, D, S), np.float64)
    for c in range(N_CORES):
        ysum += results[c]["ypT"]
    y = ysum.transpose(0, 2, 1) + np.asarray(bo, np.float32)[None, None, :]
    return np.ascontiguousarray(y.astype(np.float32))

